# revision 1
# baseline (speedup 1.0000x reference)
"""Overlapping-chunk extraction kernel for Trainium2 (Bass).

Computes out[b, j, c, f] = x[b, 125*j + c, f] for j in [0, 255), c in [0, 250),
i.e. 255 half-overlapping chunks of length 250 from a (16, 32000, 64) signal.

Strategy (pure data movement, memory-bound):
  - Shard batch across 8 cores: 2 samples per core.
  - Per sample: ONE direct HBM->HBM DMA. Source = overlapping strided view
    (255 blocks of 16000 fp32 at stride 8000). Destination = fully contiguous
    output sample. A single sequential HBM write stream is the key to
    throughput on this part; measured ~4.7x faster than SBUF-staged variants
    with strided writes.
"""

import numpy as np

import concourse.bass as bass
import concourse.mybir as mybir
from concourse.bass_utils import run_bass_kernel_spmd

# Problem shape (hardcoded per contract)
B, T, F = 16, 32000, 64
N_CORES = 8
S = B // N_CORES          # samples per core = 2
NFC = 128                 # non-overlapping chunks per sample
CHUNK = 250               # frames per chunk
NOV = 2 * NFC - 1         # 255 overlapped output chunks
PART_FREE = CHUNK * F     # 16000 fp32 per chunk
HALF_FREE = PART_FREE // 2  # 8000 fp32 = 125 frames (chunk advance)
SAMPLE_IN = T * F         # 2_048_000 fp32 per input sample
SAMPLE_OUT = NOV * PART_FREE  # 4_080_000 fp32 per output sample

_NC_CACHE = {}


def _build_module():
    nc = bass.Bass(trn_type="TRN2")
    x = nc.dram_tensor("x", [S, T, F], mybir.dt.float32, kind="ExternalInput")
    y = nc.dram_tensor(
        "y", [S, NOV, CHUNK, F], mybir.dt.float32, kind="ExternalOutput"
    )
    x_t = x[:, :, :].tensor
    y_t = y[:, :, :, :].tensor

    with (
        nc.semaphore("st") as st,
        nc.Block() as block,
    ):
        @block.gpsimd
        def _(gpsimd):
            with nc.allow_non_contiguous_dma(reason="overlapping chunk reads"):
                for s in range(S):
                    src = bass.AP(
                        x_t, s * SAMPLE_IN, [[HALF_FREE, NOV], [1, PART_FREE]]
                    )
                    dst = bass.AP(
                        y_t, s * SAMPLE_OUT, [[PART_FREE, NOV], [1, PART_FREE]]
                    )
                    gpsimd.dma_start(dst, src).then_inc(st, 16)
                gpsimd.wait_ge(st, 16 * S)

    return nc


def get_module():
    if "nc" not in _NC_CACHE:
        _NC_CACHE["nc"] = _build_module()
    return _NC_CACHE["nc"]


def kernel(x):
    x = np.ascontiguousarray(np.asarray(x), dtype=np.float32)
    assert x.shape == (B, T, F), x.shape
    nc = get_module()
    in_maps = [{"x": x[i * S : (i + 1) * S]} for i in range(N_CORES)]
    res = run_bass_kernel_spmd(nc, in_maps, core_ids=list(range(N_CORES)))
    return np.concatenate([r["y"] for r in res.results], axis=0)



# revision 2
# speedup vs baseline: 1.1650x; 1.1650x over previous
"""Overlapping-chunk extraction kernel for Trainium2 (Bass).

Computes out[b, j, c, f] = x[b, 125*j + c, f] for j in [0, 255), c in [0, 250):
255 half-overlapping chunks of length 250 from a (16, 32000, 64) fp32 signal.
Batch is sharded across 8 cores (2 samples each).

Strategy (pure data movement; probe-driven design):
  * The sustained DMA limit on this part is ~350-400 GB/s per core of TOTAL
    HBM bytes (reads + writes).  The old HBM->HBM kernel paid 2x reads
    (overlap re-read): 65.3 MB -> ~186 us.  This kernel stages through SBUF
    and hits the information-theoretic floor: input read once (16.4 MB) +
    output written once (32.8 MB) = 49.2 MB -> ~138 us measured.
  * 127-partition DMAs take a ~20x slow descriptor path; every AP here has
    exactly 128 partitions.  Odd chunks number 127 per sample, so the two
    odd-half store streams are padded to 128 blocks.  Pad blocks land in
    8000-element scratch gaps built into the per-core output buffer (layout
    [scr][sample0][scr][sample1][scr], sliced off host-side), so no store
    depends on any other store -- the only sync is load -> stores.

Per-core program (4 DMAs, all 128-partition, h_k = x el [8000k,+8000) of a
sample, s_base(s) = 8000 + 4_088_000 s in the padded output):
  LOAD  TT[q, 16000 s : +16000] <- x[s] el [16000 q : +16000]      (16.4 MB)
  E     y[s_base + 32000 q : +16000] <- TT[q, 16000 s]   even chunks (16.4 MB)
  O1    y[s_base + 16000 + 32000 q : +8000] <- h_{2q+1}  odd 1st halves
        (q=127 pad -> post-sample scratch)                          (8.2 MB)
  O2    y[s_base -  8000 + 32000 q : +8000] <- h_{2q}    odd 2nd halves
        (q=0 pad -> pre-sample scratch)                             (8.2 MB)
"""

import numpy as np

import concourse.bass as bass
import concourse.mybir as mybir
from concourse.bass_utils import run_bass_kernel_spmd

# Problem shape (hardcoded per contract)
B, T, F = 16, 32000, 64
N_CORES = 8
S = B // N_CORES              # samples per core = 2
NFC = 128                     # non-overlapping chunks per sample
CHUNK = 250                   # frames per chunk
NOV = 2 * NFC - 1             # 255 overlapped output chunks
PART_FREE = CHUNK * F         # 16000 fp32 per chunk
HALF_FREE = PART_FREE // 2    # 8000 fp32 (chunk advance)
SAMPLE_IN = T * F             # 2_048_000 fp32 per input sample
SAMPLE_OUT = NOV * PART_FREE  # 4_080_000 fp32 per output sample
SLOT = SAMPLE_OUT + HALF_FREE   # sample + shared scratch gap
YPAD = HALF_FREE + S * SLOT     # 8_184_000 fp32 padded per-core output
TT_COLS = 2 * PART_FREE         # 32000 fp32 per SBUF partition

_NC_CACHE = {}


def build_module(R=1, name="chunkop_final"):
    """Per-core Bass program; R>1 chains R copies back-to-back (for timing)."""
    nc = bass.Bass(trn_type="TRN2", name=f"{name}_r{R}")
    x = nc.dram_tensor("x", [S, T, F], mybir.dt.float32, kind="ExternalInput")
    y = nc.dram_tensor(
        "y", [YPAD // HALF_FREE, HALF_FREE], mybir.dt.float32,
        kind="ExternalOutput",
    )
    x_t = x[:, :, :].tensor
    y_t = y[:, :].tensor

    with (
        nc.sbuf_tensor([128, TT_COLS], mybir.dt.float32) as tt,
        nc.semaphore("ld") as ld,
        nc.semaphore("st") as st,
        nc.Block() as block,
    ):
        tt_t = tt[:, :].tensor

        @block.gpsimd
        def _(gpsimd):
            with nc.allow_non_contiguous_dma(reason="overlap chunk layout"):
                for r in range(R):
                    if r > 0:
                        # WAR: next load overwrites tiles read by prev stores
                        gpsimd.wait_ge(st, 48 * r)
                    gpsimd.dma_start(
                        tt[:, :],
                        bass.AP(x_t, 0,
                                [[PART_FREE, 128],
                                 [SAMPLE_IN, 2],
                                 [1, PART_FREE]]),
                    ).then_inc(ld, 16)
                    gpsimd.wait_ge(ld, 16 * (r + 1))
                    # E: even chunks
                    gpsimd.dma_start(
                        bass.AP(y_t, HALF_FREE,
                                [[2 * PART_FREE, 128], [SLOT, 2],
                                 [1, PART_FREE]]),
                        bass.AP(tt_t, 0,
                                [[TT_COLS, 128], [PART_FREE, 2],
                                 [1, PART_FREE]]),
                    ).then_inc(st, 16)
                    # O1: odd first halves (q=127 -> scratch)
                    gpsimd.dma_start(
                        bass.AP(y_t, HALF_FREE + PART_FREE,
                                [[2 * PART_FREE, 128], [SLOT, 2],
                                 [1, HALF_FREE]]),
                        bass.AP(tt_t, HALF_FREE,
                                [[TT_COLS, 128], [PART_FREE, 2],
                                 [1, HALF_FREE]]),
                    ).then_inc(st, 16)
                    # O2: odd second halves (q=0 -> scratch)
                    gpsimd.dma_start(
                        bass.AP(y_t, 0,
                                [[2 * PART_FREE, 128], [SLOT, 2],
                                 [1, HALF_FREE]]),
                        bass.AP(tt_t, 0,
                                [[TT_COLS, 128], [PART_FREE, 2],
                                 [1, HALF_FREE]]),
                    ).then_inc(st, 16)
                gpsimd.wait_ge(st, 48 * R)

    return nc


def get_module():
    if "nc" not in _NC_CACHE:
        _NC_CACHE["nc"] = build_module()
    return _NC_CACHE["nc"]


def kernel(x):
    x = np.ascontiguousarray(np.asarray(x), dtype=np.float32)
    assert x.shape == (B, T, F), x.shape
    nc = get_module()
    in_maps = [{"x": x[i * S : (i + 1) * S]} for i in range(N_CORES)]
    res = run_bass_kernel_spmd(nc, in_maps, core_ids=list(range(N_CORES)))
    out = np.empty((B, NOV, CHUNK, F), dtype=np.float32)
    for c, r in enumerate(res.results):
        flat = r["y"].reshape(-1)
        for s in range(S):
            base = HALF_FREE + s * SLOT
            out[c * S + s] = flat[base:base + SAMPLE_OUT].reshape(
                NOV, CHUNK, F
            )
    return out
